# revision 1
# baseline (speedup 1.0000x reference)
"""Trainium2 8-core tensor-parallel transformer layer — v8.

On top of the v2 pipelined design (chunked AllGathers in separate DRAM
bounce tensors, batched LN row math, LN stats folded into producer loops):
- Warmup dummy AllGather absorbs the first-collective init cost.
- bf16 LN1 stats matmuls; bf16 residual stream (ln_in/attn_sb/mlp_sb).
- Dense projection chunks interleaved into the attention loop (batch b's
  dense runs during batch b+1's attention, sharing the ctx PSUM tags).
- Attention software-pipelined by one kt iteration: all four heads'
  scores+exp are emitted a full iteration ahead of the ctx matmuls.
- All three stat AllReduces split into token-halves; AR3 for the first
  half fires mid-attention (its dense chunks finished early), so the
  LN3 -> AR2 -> LN2 -> x2-AllGather chain for half 0 starts the moment
  attention ends, and LN4 of half 0 overlaps 4hh of half 1.
"""

import os
import sys

sys.path.insert(0, "/opt/trn_rl_repo")
os.environ.setdefault("MYCRO_LOCAL_CACHE", "1")
os.environ.setdefault("JAX_PLATFORMS", "cpu,axon")

import numpy as np
import ml_dtypes

import concourse.bass as bass
import concourse.mybir as mybir
import concourse.tile as tile
from concourse import bacc
from concourse.bass_utils import run_bass_kernel_spmd

F32 = mybir.dt.float32
BF16 = mybir.dt.bfloat16
AF = mybir.ActivationFunctionType
ALU = mybir.AluOpType

P = 128
B, S, H, NH = 4, 1024, 2048, 32
HD = H // NH
T = B * S
NC = 8
HPC = NH // NC                 # 4 heads/core
DC = H // NC                   # 256
FC = 4 * H // NC               # 1024
F4 = 4 * H                     # 8192
TC = 512
NTC = T // TC                  # 8
NFC = H // P                   # 16
EPS = 1e-5
RG = [list(range(NC))]

bf16 = ml_dtypes.bfloat16


def _causal_block_status(mask2d):
    mt = mask2d.T
    status = {}
    for kt in range(S // P):
        for qc in range(S // TC):
            blk = mt[kt * P:(kt + 1) * P, qc * TC:(qc + 1) * TC]
            if np.all(blk == 0):
                status[(kt, qc)] = "skip"
            elif np.all(blk == 1):
                status[(kt, qc)] = "full"
            else:
                status[(kt, qc)] = "masked"
    return status


def _evict(nc, dst, ps, bias_ap, zero_bias):
    if zero_bias:
        nc.scalar.activation(dst, ps, AF.Copy)
    else:
        nc.scalar.activation(dst, ps, AF.Identity, bias=bias_ap)


def build_program(block_status, zero_bv=True, zero_bias=True):
    nc = bacc.Bacc("TRN2", target_bir_lowering=False, debug=False,
                   num_devices=NC)

    def register_const_ap(dtype, value):
        t = nc.alloc_sbuf_tensor(f"const-{dtype.name}-{value}", [128, 1], dtype)
        nc.gpsimd.memset(t.ap(), value)
        nc.const_aps.aps[(dtype, value)] = t.ap()

    register_const_ap(F32, EPS)
    register_const_ap(F32, float(1.0 / np.sqrt(HD)))
    nc.all_engine_barrier()

    # ---------------- DRAM I/O (same contract as v1) ----------------
    h_ln1 = nc.dram_tensor("h_ln1", [H, TC], F32, kind="ExternalInput")
    h_res = nc.dram_tensor("h_res", [DC, T], F32, kind="ExternalInput")
    ln1_w = nc.dram_tensor("ln1_w", [H, 1], F32, kind="ExternalInput")
    ln1_b = nc.dram_tensor("ln1_b", [H, 1], F32, kind="ExternalInput")
    ln2_w = nc.dram_tensor("ln2_w", [DC, 1], F32, kind="ExternalInput")
    ln2_b = nc.dram_tensor("ln2_b", [DC, 1], F32, kind="ExternalInput")
    ln3_w = nc.dram_tensor("ln3_w", [DC, 1], F32, kind="ExternalInput")
    ln3_b = nc.dram_tensor("ln3_b", [DC, 1], F32, kind="ExternalInput")
    ln4_w = nc.dram_tensor("ln4_w", [DC, 1], F32, kind="ExternalInput")
    ln4_b = nc.dram_tensor("ln4_b", [DC, 1], F32, kind="ExternalInput")
    w_qkv = nc.dram_tensor("w_qkv", [H, 3 * DC], BF16, kind="ExternalInput")
    b_qk = nc.dram_tensor("b_qk", [2 * DC, 1], F32, kind="ExternalInput")
    b_v = nc.dram_tensor("b_v", [1, DC], F32, kind="ExternalInput")
    w_dense = nc.dram_tensor("w_dense", [H, DC], BF16, kind="ExternalInput")
    b_dense = nc.dram_tensor("b_dense", [DC, 1], F32, kind="ExternalInput")
    w_h4h = nc.dram_tensor("w_h4h", [H, FC], BF16, kind="ExternalInput")
    b_h4h = nc.dram_tensor("b_h4h", [FC, 1], F32, kind="ExternalInput")
    w_4hh = nc.dram_tensor("w_4hh", [F4, DC], BF16, kind="ExternalInput")
    b_4hh = nc.dram_tensor("b_4hh", [DC, 1], F32, kind="ExternalInput")
    maskT = nc.dram_tensor("maskT", [S, S], BF16, kind="ExternalInput")
    out_ext = nc.dram_tensor("out", [DC, T], F32, kind="ExternalOutput")

    masked_blocks = sorted(k for k, v in block_status.items() if v == "masked")
    mask_slot = {blk: i for i, blk in enumerate(masked_blocks)}

    with tile.TileContext(nc) as tc:
        with tc.tile_pool(name="const", bufs=1) as const, \
             tc.tile_pool(name="resid", bufs=1) as resid, \
             tc.tile_pool(name="dram", bufs=1, space="DRAM") as dram:

            # ---------- constants ----------
            ones_f = const.tile([P, 1], F32)
            nc.vector.memset(ones_f[:, :], 1.0)
            ones_bf = const.tile([P, 1], BF16)
            nc.vector.memset(ones_bf[:, :], 1.0)
            ones_rows_bf = const.tile([P, P], BF16)
            nc.vector.memset(ones_rows_bf[:, :], 1.0)

            ln1w_sb = const.tile([P, NFC], F32)
            ln1b_sb = const.tile([P, NFC], F32)
            for fc in range(NFC):
                nc.sync.dma_start(out=ln1w_sb[:, fc:fc + 1],
                                  in_=ln1_w[fc * P:(fc + 1) * P, 0:1])
                nc.sync.dma_start(out=ln1b_sb[:, fc:fc + 1],
                                  in_=ln1_b[fc * P:(fc + 1) * P, 0:1])

            cpack = const.tile([P, 28], F32)
            _cofs = [0]

            def load_cols(t, ncols=2):
                base = _cofs[0]
                _cofs[0] += ncols
                for m in range(ncols):
                    nc.sync.dma_start(out=cpack[:, base + m:base + m + 1],
                                      in_=t[m * P:(m + 1) * P, 0:1])
                return cpack[:, base:base + ncols]

            ln2w_sb = load_cols(ln2_w)
            ln2b_sb = load_cols(ln2_b)
            ln3w_sb = load_cols(ln3_w)
            ln3b_sb = load_cols(ln3_b)
            ln4w_sb = load_cols(ln4_w)
            ln4b_sb = load_cols(ln4_b)
            bdense_sb = load_cols(b_dense)
            b4hh_sb = load_cols(b_4hh)
            bqk_sb = load_cols(b_qk, 4)
            bh4h_sb = load_cols(b_h4h, 8)

            if not zero_bv:
                bv_row = const.tile([1, DC], F32)
                nc.sync.dma_start(out=bv_row[:, :], in_=b_v[0:1, :])
                bv_b = const.tile([P, DC], F32)
                nc.gpsimd.partition_broadcast(bv_b[:, :], bv_row[:, :])

            if masked_blocks:
                mask_sb = const.tile([P, len(masked_blocks) * TC], BF16)
                for (kt, qc), i in mask_slot.items():
                    nc.sync.dma_start(
                        out=mask_sb[:, i * TC:(i + 1) * TC],
                        in_=maskT[kt * P:(kt + 1) * P, qc * TC:(qc + 1) * TC])

            # ---------- residents ----------
            ln_in = [resid.tile([P, T], BF16, name=f"ln_in{m}")
                     for m in range(2)]
            attn_sb = [resid.tile([P, T], BF16, tag="colsAM", bufs=2,
                                  name=f"attn_sb{m}") for m in range(2)]

            # ---------- DRAM bounces (chunked) ----------
            # x1: 2 feature-halves [128, 8*TC] each
            ag_x1_in = [dram.tile([P, 8 * TC], BF16, name=f"agx1i{h}")
                        for h in range(2)]
            ag_x1_out = [dram.tile([NC * P, 8 * TC], BF16,
                                   addr_space="Shared", name=f"agx1o{h}")
                         for h in range(2)]
            # ctx: per-batch chunks, free = qc*2*TC... chunk layout
            # [128, 2048]: (qc, pair) -> qc*1024 + pair*512
            ag_ctx_in = [dram.tile([P, 2 * S], BF16, name=f"agcxi{b}")
                         for b in range(B)]
            ag_ctx_out = [dram.tile([NC * P, 2 * S], BF16,
                                    addr_space="Shared", name=f"agcxo{b}")
                          for b in range(B)]
            # x2: 2 token-halves, free = t8r*1024 + m*512 (t8r in 0..3)
            ag_x2_in = [dram.tile([P, 2 * 1024], BF16, name=f"agx2i{h}")
                        for h in range(4)]
            ag_x2_out = [dram.tile([NC * P, 2 * 1024], BF16,
                                   addr_space="Shared", name=f"agx2o{h}")
                         for h in range(4)]
            # inter: per-t8 chunks, free = m*512 (m in 0..7)
            ag_int_in = [dram.tile([P, 8 * TC], BF16, name=f"agini{k}")
                         for k in range(NTC)]
            ag_int_out = [dram.tile([NC * P, 8 * TC], BF16,
                                    addr_space="Shared", name=f"agino{k}")
                          for k in range(NTC)]
            ar3h_in = [dram.tile([2, T // 2], F32, name=f"ar3i{i}")
                       for i in range(2)]
            ar3h_out = [dram.tile([2, T // 2], F32, addr_space="Shared",
                                  name=f"ar3o{i}") for i in range(2)]
            ar2h_in = [dram.tile([2, T // 2], F32, name=f"ar2i{i}")
                       for i in range(2)]
            ar2h_out = [dram.tile([2, T // 2], F32, addr_space="Shared",
                                  name=f"ar2o{i}") for i in range(2)]
            ar4h_in = [dram.tile([2, T // 2], F32, name=f"ar4i{i}")
                       for i in range(2)]
            ar4h_out = [dram.tile([2, T // 2], F32, addr_space="Shared",
                                  name=f"ar4o{i}") for i in range(2)]

            warm_in = dram.tile([1, 64], BF16, name="warm_in")
            warm_out = dram.tile([NC, 64], BF16, addr_space="Shared",
                                 name="warm_out")
            warm_sb = const.tile([1, 64], BF16)
            nc.vector.memset(warm_sb[:, :], 0.0)
            nc.sync.dma_start(out=warm_in[:, :], in_=warm_sb[:, :])
            nc.gpsimd.collective_compute(
                "AllGather", ALU.bypass, replica_groups=RG,
                ins=[warm_in[:, :].opt()], outs=[warm_out[:, :].opt()])

            # =========================================================
            # Phase A: LN1 -> x1 (bf16) -> 2 half AllGathers
            # =========================================================
            with tc.tile_pool(name="ph_a", bufs=1) as pha, \
                 tc.tile_pool(name="ph_a_ps", bufs=2, space="PSUM") as phaps:
                h1 = [pha.tile([P, TC], F32, name=f"h1_{fc}")
                      for fc in range(NFC)]
                for fc in range(NFC):
                    nc.sync.dma_start(out=h1[fc][:, :],
                                      in_=h_ln1[fc * P:(fc + 1) * P, :])
                ps_s = phaps.tile([1, TC], F32, name="ps_s")
                ps_q = phaps.tile([1, TC], F32, name="ps_q")
                for fc in range(NFC):
                    h1b = pha.tile([P, TC], BF16, tag="h1b", bufs=3,
                                   name="h1b")
                    nc.vector.tensor_copy(h1b[:, :], h1[fc][:, :])
                    nc.tensor.matmul(ps_s[:, :], ones_bf[:, 0:1], h1b[:, :],
                                     start=(fc == 0), stop=(fc == NFC - 1))
                    sq = pha.tile([P, TC], BF16, tag="sq", bufs=3, name="sq")
                    nc.vector.tensor_mul(sq[:, :], h1b[:, :], h1b[:, :])
                    nc.tensor.matmul(ps_q[:, :], ones_bf[:, 0:1], sq[:, :],
                                     start=(fc == 0), stop=(fc == NFC - 1))
                mu = pha.tile([1, TC], F32)
                m2 = pha.tile([1, TC], F32)
                var = pha.tile([1, TC], F32)
                sd = pha.tile([1, TC], F32)
                a_row = pha.tile([1, TC], F32)
                b2_row = pha.tile([1, TC], F32)
                nc.vector.tensor_scalar_mul(mu[:, :], ps_s[:, :], 1.0 / H)
                nc.vector.tensor_scalar_mul(m2[:, :], ps_q[:, :], 1.0 / H)
                nc.vector.tensor_mul(var[:, :], mu[:, :], mu[:, :])
                nc.vector.tensor_sub(var[:, :], m2[:, :], var[:, :])
                nc.scalar.activation(sd[:, :], var[:, :], AF.Sqrt, bias=EPS)
                nc.vector.reciprocal(a_row[:, :], sd[:, :])
                nc.vector.tensor_mul(b2_row[:, :], mu[:, :], a_row[:, :])
                nc.vector.tensor_scalar_mul(b2_row[:, :], b2_row[:, :], -1.0)
                a_b = pha.tile([P, TC], F32)
                b2_b = pha.tile([P, TC], F32)
                nc.gpsimd.partition_broadcast(a_b[:, :], a_row[:, :])
                nc.gpsimd.partition_broadcast(b2_b[:, :], b2_row[:, :])
                x1h = [pha.tile([P, 8 * TC], BF16, name=f"x1h{h}")
                       for h in range(2)]
                for fc in range(NFC):
                    t1 = pha.tile([P, TC], F32, tag="t1", bufs=3, name="t1")
                    nc.vector.tensor_mul(t1[:, :], h1[fc][:, :], a_b[:, :])
                    nc.vector.tensor_add(t1[:, :], t1[:, :], b2_b[:, :])
                    hh, fr = fc // 8, fc % 8
                    nc.vector.tensor_scalar(
                        x1h[hh][:, fr * TC:(fr + 1) * TC], t1[:, :],
                        ln1w_sb[:, fc:fc + 1], ln1b_sb[:, fc:fc + 1],
                        ALU.mult, ALU.add)
                    if fr == 7:
                        nc.sync.dma_start(out=ag_x1_in[hh][:, :],
                                          in_=x1h[hh][:, :])

            for hh in range(2):
                nc.gpsimd.collective_compute(
                    "AllGather", ALU.bypass, replica_groups=RG,
                    ins=[ag_x1_in[hh][:, :].opt()],
                    outs=[ag_x1_out[hh][:, :].opt()])

            # =========================================================
            # Phase B: QKV (consumes x1 halves as they arrive)
            # =========================================================
            phd_w_cm = tc.tile_pool(name="ph_d_w", bufs=1)
            phdw = phd_w_cm.__enter__()
            phd_cm = tc.tile_pool(name="ph_d", bufs=1)
            phd = phd_cm.__enter__()
            attn_res_cm = tc.tile_pool(name="attn_res", bufs=1)
            attn_res = attn_res_cm.__enter__()
            qT2 = attn_res.tile([P, 2 * T], BF16)
            kT2 = attn_res.tile([P, 2 * T], BF16)
            v_sb = attn_res.tile([P, (T // P) * DC], BF16)
            with tc.tile_pool(name="ph_b_w", bufs=1) as phbw, \
                 tc.tile_pool(name="ph_b", bufs=2) as phb, \
                 tc.tile_pool(name="ph_b_ps", bufs=3, space="PSUM") as phbps:
                wq_all = phbw.tile([P, NFC * 3 * DC], BF16, name="wq_all")
                for fc in range(NFC):
                    nc.sync.dma_start(
                        out=wq_all[:, fc * 3 * DC:(fc + 1) * 3 * DC],
                        in_=w_qkv[fc * P:(fc + 1) * P, :])
                for t8 in range(NTC):
                    x1c = [phb.tile([P, 8 * TC], BF16, tag=f"x1c{h}",
                                    name=f"x1c{h}") for h in range(2)]
                    for hh in range(2):
                        nc.sync.dma_start(
                            out=x1c[hh][:, :],
                            in_=ag_x1_out[hh][t8 * P:(t8 + 1) * P, :])

                    def xs(fc, lo, sz):
                        hh, fr = fc // 8, fc % 8
                        return x1c[hh][:, fr * TC + lo: fr * TC + lo + sz]

                    for m in range(4):
                        ps = phbps.tile([P, TC], F32, tag="qk", name="ps_qk")
                        for fc in range(NFC):
                            nc.tensor.matmul(
                                ps[:, :],
                                wq_all[:, fc * 3 * DC + m * P:
                                       fc * 3 * DC + (m + 1) * P],
                                xs(fc, 0, TC),
                                start=(fc == 0), stop=(fc == NFC - 1))
                        dst = qT2 if m < 2 else kT2
                        pair = m % 2
                        off = pair * T + t8 * TC
                        _evict(nc, dst[:, off:off + TC], ps[:, :],
                               bqk_sb[:, m:m + 1], zero_bias)
                    for tt in range(TC // P):
                        psv = phbps.tile([P, DC], F32, tag="v", name="ps_v")
                        for fc in range(NFC):
                            nc.tensor.matmul(
                                psv[:, :], xs(fc, tt * P, P),
                                wq_all[:, fc * 3 * DC + 2 * DC:
                                       fc * 3 * DC + 3 * DC],
                                start=(fc == 0), stop=(fc == NFC - 1))
                        ttg = t8 * (TC // P) + tt
                        voff = ttg * DC
                        if zero_bv:
                            nc.scalar.activation(v_sb[:, voff:voff + DC],
                                                 psv[:, :], AF.Copy)
                        else:
                            nc.vector.tensor_add(v_sb[:, voff:voff + DC],
                                                 psv[:, :], bv_b[:, :])

            # =========================================================
            # Phase C: attention; ctxF free = (qc*1024 + pair*512) per b
            # =========================================================
            ctx_cm = tc.tile_pool(name="ctx_pool", bufs=1)
            ctx_pool = ctx_cm.__enter__()
            ctxF = [ctx_pool.tile([P, 2 * S], BF16, name=f"ctxF{b}")
                    for b in range(B)]
            wd_all = phdw.tile([P, NFC * DC], BF16, name="wd_all")
            for fc in range(NFC):
                nc.sync.dma_start(out=wd_all[:, fc * DC:(fc + 1) * DC],
                                  in_=w_dense[fc * P:(fc + 1) * P, :])
            with tc.tile_pool(name="ph_c", bufs=1) as phc, \
                 tc.tile_pool(name="ph_c_ps", bufs=1, space="PSUM") as phcps:

                def dense_chunk(t8):
                    b_, qc_ = t8 // 2, t8 % 2
                    cx_t = [phd.tile([P, TC], BF16, tag="cx", bufs=18,
                                     name=f"cx{q}") for q in range(NFC)]
                    for c8 in range(NC):
                        for p2 in range(2):
                            fc = c8 * 2 + p2
                            nc.sync.dma_start(
                                out=cx_t[fc][:, :],
                                in_=ag_ctx_out[b_][
                                    c8 * P:(c8 + 1) * P,
                                    qc_ * 2 * TC + p2 * TC:
                                    qc_ * 2 * TC + (p2 + 1) * TC])
                    dps = [phcps.tile([P, TC], F32, tag=f"ctx{m}", bufs=1,
                                      name=f"ps_d{m}") for m in range(2)]
                    for fc in range(NFC):
                        for m in range(2):
                            nc.tensor.matmul(
                                dps[m][:, :],
                                wd_all[:, fc * DC + m * P:
                                       fc * DC + (m + 1) * P],
                                cx_t[fc][:, :],
                                start=(fc == 0), stop=(fc == NFC - 1))
                    for m in range(2):
                        _evict(nc, attn_sb[m][:, t8 * TC:(t8 + 1) * TC],
                               dps[m][:, :], bdense_sb[:, m:m + 1],
                               zero_bias)
                    _stats_t8(nc, phd, phcps, attn_sb, t8,
                              ar3h_in[t8 // 4], ones_bf, stag="s", qtag="s",
                              sbufs=4, slot=t8 % 4)

                for b in range(B):
                    for qc in range(S // TC):
                        ctx_ps = [phcps.tile([P, TC], F32, tag=f"ctx{p}",
                                             bufs=1, name=f"ctx_ps{p}")
                                  for p in range(2)]
                        den_ps = phcps.tile([P, TC], F32, tag="den",
                                            bufs=1, name="den_ps")
                        kts = [kt for kt in range(S // P)
                               if block_status[(kt, qc)] != "skip"]
                        nkt = len(kts)

                        def emit_scores(ki):
                            kt = kts[ki]
                            st = block_status[(kt, qc)]
                            es = []
                            for h in range(HPC):
                                pair, rho = h // 2, h % 2
                                ps_s = phcps.tile([P, TC], F32, tag="s",
                                                  bufs=4, name="ps_s")
                                qoff = pair * T + b * S + qc * TC
                                koff = pair * T + b * S + kts[ki] * P
                                nc.tensor.matmul(
                                    ps_s[:, :],
                                    kT2[rho * HD:(rho + 1) * HD,
                                        koff:koff + P],
                                    qT2[rho * HD:(rho + 1) * HD,
                                        qoff:qoff + TC],
                                    start=True, stop=True)
                                e = phc.tile([P, TC], BF16, tag="e", bufs=10,
                                             name="e")
                                nc.scalar.activation(e[:, :], ps_s[:, :],
                                                     AF.Exp,
                                                     scale=1.0 / np.sqrt(HD))
                                if st == "masked":
                                    i = mask_slot[(kt, qc)]
                                    nc.vector.tensor_mul(
                                        e[:, :], e[:, :],
                                        mask_sb[:, i * TC:(i + 1) * TC])
                                es.append(e)
                            return es

                        def emit_ctx(ki, es):
                            kt = kts[ki]
                            ttg = b * (S // P) + kt
                            for h in range(HPC):
                                pair, rho = h // 2, h % 2
                                nc.tensor.matmul(
                                    ctx_ps[pair][rho * HD:(rho + 1) * HD, :],
                                    v_sb[:, ttg * DC + h * HD:
                                         ttg * DC + (h + 1) * HD],
                                    es[h][:, :],
                                    start=(ki == 0), stop=(ki == nkt - 1))
                                nc.tensor.matmul(
                                    den_ps[32 * h:32 * h + 1, :],
                                    ones_bf[:, 0:1], es[h][:, :],
                                    start=(ki == 0), stop=(ki == nkt - 1),
                                    tile_position=(0, 32 * h))

                        prev = emit_scores(0)
                        for ki in range(1, nkt):
                            cur = emit_scores(ki)
                            emit_ctx(ki - 1, prev)
                            prev = cur
                        emit_ctx(nkt - 1, prev)
                        # batched reciprocal of all 4 head denominators
                        rd = phc.tile([P, TC], F32, tag="rd", bufs=2,
                                      name="rd")
                        rd_bf = phc.tile([P, TC], BF16, tag="rd_bf", bufs=2,
                                         name="rd_bf")
                        nc.vector.reciprocal(rd[:, :], den_ps[:, :])
                        nc.vector.tensor_copy(rd_bf[:, :], rd[:, :])
                        for h in range(HPC):
                            pair, rho = h // 2, h % 2
                            r32 = slice(32 * h, 32 * h + 1)
                            ps_b = phcps.tile([P, TC], F32, tag="s", bufs=4,
                                              name="ps_b")
                            nc.tensor.matmul(ps_b[:, :],
                                             ones_rows_bf[r32, :],
                                             rd_bf[r32, :], start=True,
                                             stop=True,
                                             tile_position=(32 * h, 0))
                            rd_b = phc.tile([P, TC], F32, tag="rd_b", bufs=2,
                                            name="rd_b")
                            nc.vector.tensor_copy(rd_b[:, :], ps_b[:, :])
                            off = qc * 2 * TC + pair * TC
                            hs = slice(rho * HD, (rho + 1) * HD)
                            nc.vector.tensor_mul(ctxF[b][hs, off:off + TC],
                                                 ctx_ps[pair][hs, :],
                                                 rd_b[hs, :])
                        if b > 0:
                            dense_chunk(2 * (b - 1) + qc)
                        if b == 2 and qc == 1:
                            nc.gpsimd.collective_compute(
                                "AllReduce", ALU.add, replica_groups=RG,
                                ins=[ar3h_in[0][:, :].opt()],
                                outs=[ar3h_out[0][:, :].opt()])
                    nc.sync.dma_start(out=ag_ctx_in[b][:, :],
                                      in_=ctxF[b][:, :])
                    nc.gpsimd.collective_compute(
                        "AllGather", ALU.bypass, replica_groups=RG,
                        ins=[ag_ctx_in[b][:, :].opt()],
                        outs=[ag_ctx_out[b][:, :].opt()])
                for qc in range(2):
                    dense_chunk(6 + qc)
                nc.gpsimd.collective_compute(
                    "AllReduce", ALU.add, replica_groups=RG,
                    ins=[ar3h_in[1][:, :].opt()],
                    outs=[ar3h_out[1][:, :].opt()])
            ctx_cm.__exit__(None, None, None)
            attn_res_cm.__exit__(None, None, None)
            phd_ps_cm = tc.tile_pool(name="ph_d_ps", bufs=1, space="PSUM")
            phdps = phd_ps_cm.__enter__()

            if True:
                with tc.tile_pool(name="ph_d2", bufs=2) as phd2:
                    x2 = [phd2.tile([P, 2 * 1024], BF16, tag=f"x2_{h}",
                                    bufs=1, name=f"x2_{h}") for h in range(4)]
                    for half in range(2):
                        ab3 = _ln_rows_batch(nc, phd2, ar3h_out[half],
                                             f"ln3h{half}", nrows=4)
                        for t8 in range(4 * half, 4 * half + 4):
                            a_b, b2_b = _ln_bcast(nc, phd2, ab3, t8 % 4)
                            for m in range(2):
                                hres = phd2.tile([P, TC], F32, tag="hres",
                                                 bufs=8, name="hres")
                                nc.sync.dma_start(
                                    out=hres[:, :],
                                    in_=h_res[m * P:(m + 1) * P,
                                              t8 * TC:(t8 + 1) * TC])
                                sl = slice(t8 * TC, (t8 + 1) * TC)
                                t1 = phd2.tile([P, TC], F32, tag="t1",
                                               name="t1")
                                t2 = phd2.tile([P, TC], F32, tag="t2",
                                               name="t2")
                                nc.vector.tensor_mul(t1[:, :],
                                                     attn_sb[m][:, sl],
                                                     a_b[:, :])
                                nc.vector.tensor_add(t1[:, :], t1[:, :],
                                                     b2_b[:, :])
                                nc.vector.tensor_scalar(t2[:, :], t1[:, :],
                                                        ln3w_sb[:, m:m + 1],
                                                        ln3b_sb[:, m:m + 1],
                                                        ALU.mult, ALU.add)
                                nc.vector.tensor_add(ln_in[m][:, sl],
                                                     t2[:, :], hres[:, :])
                            _stats_t8(nc, phd2, phdps, ln_in, t8,
                                      ar2h_in[half], ones_bf, slot=t8 % 4)
                        nc.gpsimd.collective_compute(
                            "AllReduce", ALU.add, replica_groups=RG,
                            ins=[ar2h_in[half][:, :].opt()],
                            outs=[ar2h_out[half][:, :].opt()])
                        ab2 = _ln_rows_batch(nc, phd2, ar2h_out[half],
                                             f"ln2h{half}", nrows=4)
                        for t8 in range(4 * half, 4 * half + 4):
                            a_b, b2_b = _ln_bcast(nc, phd2, ab2, t8 % 4)
                            t8r = t8 % 4
                            for m in range(2):
                                sl = slice(t8 * TC, (t8 + 1) * TC)
                                t1 = phd2.tile([P, TC], F32, tag="t1",
                                               name="t1")
                                nc.vector.tensor_mul(t1[:, :],
                                                     ln_in[m][:, sl],
                                                     a_b[:, :])
                                nc.vector.tensor_add(t1[:, :], t1[:, :],
                                                     b2_b[:, :])
                                qq, tq = t8 // 2, t8 % 2
                                nc.vector.tensor_scalar(
                                    x2[qq][:, tq * 1024 + m * TC:
                                           tq * 1024 + (m + 1) * TC],
                                    t1[:, :], ln2w_sb[:, m:m + 1],
                                    ln2b_sb[:, m:m + 1], ALU.mult, ALU.add)
                            if t8 % 2 == 1:
                                qq = t8 // 2
                                nc.sync.dma_start(out=ag_x2_in[qq][:, :],
                                                  in_=x2[qq][:, :])
                                nc.gpsimd.collective_compute(
                                    "AllGather", ALU.bypass,
                                    replica_groups=RG,
                                    ins=[ag_x2_in[qq][:, :].opt()],
                                    outs=[ag_x2_out[qq][:, :].opt()])

            phd_cm.__exit__(None, None, None)
            phd_w_cm.__exit__(None, None, None)

            # =========================================================
            # Phase E+F: h4h+gelu -> per-t8 inter AGs -> 4hh (+LN4 stats)
            # one shared PSUM pool: h4h groups (4) + 4hh (2) + stats (2)
            # =========================================================
            with tc.tile_pool(name="ph_e_w", bufs=1) as phew, \
                 tc.tile_pool(name="ph_ef", bufs=1) as phef, \
                 tc.tile_pool(name="ph_ef_ps", bufs=1, space="PSUM") as pheps:
                whp_cm = tc.tile_pool(name="ph_wh", bufs=1)
                whp = whp_cm.__enter__()
                wh_all = whp.tile([P, NFC * FC], BF16, name="wh_all")
                for fc in range(NFC):
                    nc.sync.dma_start(out=wh_all[:, fc * FC:(fc + 1) * FC],
                                      in_=w_h4h[fc * P:(fc + 1) * P, :])
                w4_all = phew.tile([P, (F4 // P) * DC], BF16,
                                   name="w4_all")
                for j in range(F4 // P):
                    nc.sync.dma_start(out=w4_all[:, j * DC:(j + 1) * DC],
                                      in_=w_4hh[j * P:(j + 1) * P, :])
                mlp_sb = [resid.tile([P, T], BF16, tag="colsAM", bufs=2,
                                     name=f"mlp_sb{m}") for m in range(2)]
                # h4h producer per t8 chunk
                for t8 in range(NTC):
                    hh, t8r = t8 // 2, t8 % 2
                    x2c_all = phef.tile([P, NFC * TC], BF16, tag="x2c",
                                        bufs=2, name="x2c_all")
                    for c8 in range(NC):
                        for m2 in range(2):
                            fc = c8 * 2 + m2
                            nc.sync.dma_start(
                                out=x2c_all[:, fc * TC:(fc + 1) * TC],
                                in_=ag_x2_out[hh][c8 * P:(c8 + 1) * P,
                                                  t8r * 1024 + m2 * TC:
                                                  t8r * 1024 + (m2 + 1) * TC])
                    for g in range(4):
                        it = phef.tile([P, 2 * TC], BF16, tag="it", bufs=2,
                                       name="it")
                        ps = [pheps.tile([P, TC], F32, tag=f"h{mi}", bufs=1,
                                         name=f"ps_h{mi}") for mi in range(2)]
                        for fc in range(NFC):
                            for mi in range(2):
                                m = g * 2 + mi
                                nc.tensor.matmul(
                                    ps[mi][:, :],
                                    wh_all[:, fc * FC + m * P:
                                           fc * FC + (m + 1) * P],
                                    x2c_all[:, fc * TC:(fc + 1) * TC],
                                    start=(fc == 0), stop=(fc == NFC - 1))
                        for mi in range(2):
                            nc.scalar.activation(
                                it[:, mi * TC:(mi + 1) * TC], ps[mi][:, :],
                                AF.Gelu_apprx_tanh,
                                bias=bh4h_sb[:, g * 2 + mi:g * 2 + mi + 1])
                        nc.sync.dma_start(
                            out=ag_int_in[t8][:, g * 2 * TC:(g + 1) * 2 * TC],
                            in_=it[:, :])
                    nc.gpsimd.collective_compute(
                        "AllGather", ALU.bypass, replica_groups=RG,
                        ins=[ag_int_in[t8][:, :].opt()],
                        outs=[ag_int_out[t8][:, :].opt()])
                whp_cm.__exit__(None, None, None)
                # 4hh consumer per t8 chunk
                for t8 in range(NTC):
                    ps = [pheps.tile([P, TC], F32, tag=f"f{m}", bufs=1,
                                     name=f"ps_f{m}") for m in range(2)]
                    for j in range(F4 // P):
                        c8, m8 = j // 8, j % 8
                        i4 = phef.tile([P, TC], BF16, tag="i4", bufs=12,
                                       name="i4")
                        nc.sync.dma_start(
                            out=i4[:, :],
                            in_=ag_int_out[t8][c8 * P:(c8 + 1) * P,
                                               m8 * TC:(m8 + 1) * TC])
                        for m in range(2):
                            nc.tensor.matmul(
                                ps[m][:, :],
                                w4_all[:, j * DC + m * P:
                                       j * DC + (m + 1) * P],
                                i4[:, :],
                                start=(j == 0), stop=(j == F4 // P - 1))
                    for m in range(2):
                        _evict(nc, mlp_sb[m][:, t8 * TC:(t8 + 1) * TC],
                               ps[m][:, :], b4hh_sb[:, m:m + 1], zero_bias)
                    _stats_t8(nc, phef, pheps, mlp_sb, t8,
                              ar4h_in[t8 // 4], ones_bf, slot=t8 % 4)
                    if t8 == 3 or t8 == 7:
                        ih = t8 // 4
                        nc.gpsimd.collective_compute(
                            "AllReduce", ALU.add, replica_groups=RG,
                            ins=[ar4h_in[ih][:, :].opt()],
                            outs=[ar4h_out[ih][:, :].opt()])

                # LN4 normalize + final residual -> out
                with tc.tile_pool(name="ph_f2", bufs=1) as phf2:
                    ab4h = [None, None]
                    for t8 in range(NTC):
                        if t8 % 4 == 0:
                            ab4h[t8 // 4] = _ln_rows_batch(
                                nc, phf2, ar4h_out[t8 // 4], f"ln4h{t8 // 4}",
                                nrows=4)
                        a_b, b2_b = _ln_bcast(nc, phf2, ab4h[t8 // 4],
                                              t8 % 4)
                        for m in range(2):
                            sl = slice(t8 * TC, (t8 + 1) * TC)
                            t1 = phf2.tile([P, TC], F32, tag="t1", name="t1")
                            t2 = phf2.tile([P, TC], F32, tag="t2", name="t2")
                            nc.vector.tensor_mul(t1[:, :], mlp_sb[m][:, sl],
                                                 a_b[:, :])
                            nc.vector.tensor_add(t1[:, :], t1[:, :],
                                                 b2_b[:, :])
                            nc.vector.tensor_scalar(t2[:, :], t1[:, :],
                                                    ln4w_sb[:, m:m + 1],
                                                    ln4b_sb[:, m:m + 1],
                                                    ALU.mult, ALU.add)
                            ot = phf2.tile([P, TC], F32, tag="ot", name="ot")
                            nc.vector.tensor_add(ot[:, :], t2[:, :],
                                                 ln_in[m][:, sl])
                            nc.sync.dma_start(
                                out=out_ext[m * P:(m + 1) * P,
                                            t8 * TC:(t8 + 1) * TC],
                                in_=ot[:, :])
            phd_ps_cm.__exit__(None, None, None)

    nc.compile()
    return nc


def _stats_t8(nc, pool, pspool, rows, t8, ar_in, ones_bf,
              stag="st_s", qtag="st_q", sbufs=1, slot=None):
    """Sum & sumsq over the 256 local features of token-chunk t8 (bf16)."""
    if slot is None:
        slot = t8
    ps_s = pspool.tile([1, TC], F32, tag=stag, bufs=sbufs, name="ps_s")
    ps_q = pspool.tile([1, TC], F32, tag=qtag, bufs=sbufs, name="ps_q")
    sl = slice(t8 * TC, (t8 + 1) * TC)
    osl = slice(slot * TC, (slot + 1) * TC)
    for m in range(2):
        nc.tensor.matmul(ps_s[:, :], ones_bf[:, 0:1], rows[m][:, sl],
                         start=(m == 0), stop=(m == 1))
    for m in range(2):
        sq = pool.tile([P, TC], BF16, tag="sq", bufs=2, name="sq")
        nc.vector.tensor_mul(sq[:, :], rows[m][:, sl], rows[m][:, sl])
        nc.tensor.matmul(ps_q[:, :], ones_bf[:, 0:1], sq[:, :],
                         start=(m == 0), stop=(m == 1))
    tmp_s = pool.tile([1, TC], F32, tag="tmp_s", bufs=1, name="tmp_s")
    tmp_q = pool.tile([1, TC], F32, tag="tmp_q", bufs=1, name="tmp_q")
    nc.vector.tensor_copy(tmp_s[:, :], ps_s[:, :])
    nc.vector.tensor_copy(tmp_q[:, :], ps_q[:, :])
    nc.sync.dma_start(out=ar_in[0:1, osl], in_=tmp_s[:, :])
    nc.sync.dma_start(out=ar_in[1:2, osl], in_=tmp_q[:, :])


def _ln_rows_batch(nc, pool, ar_out, name, nrows=8):
    """Batched LN row math on [nrows,TC] tiles, one reciprocal total."""
    s8 = pool.tile([nrows, TC], F32, tag="lnrb_s8", bufs=1, name=f"{name}_s8")
    q8 = pool.tile([nrows, TC], F32, tag="lnrb_q8", bufs=1, name=f"{name}_q8")
    nc.sync.dma_start(out=s8[:, :], in_=ar_out[0:1, :])
    nc.sync.dma_start(out=q8[:, :], in_=ar_out[1:2, :])
    mu = pool.tile([nrows, TC], F32, tag="lnrb_mu", bufs=1, name=f"{name}_mu")
    m2 = pool.tile([nrows, TC], F32, tag="lnrb_m2", bufs=1, name=f"{name}_m2")
    var = pool.tile([nrows, TC], F32, tag="lnrb_var", bufs=1,
                    name=f"{name}_var")
    sd = pool.tile([nrows, TC], F32, tag="lnrb_sd", bufs=1, name=f"{name}_sd")
    a8 = pool.tile([nrows, TC], F32, tag="lnrb_a8", bufs=1, name=f"{name}_a8")
    b28 = pool.tile([nrows, TC], F32, tag="lnrb_b28", bufs=1,
                    name=f"{name}_b28")
    nc.vector.tensor_scalar_mul(mu[:, :], s8[:, :], 1.0 / H)
    nc.vector.tensor_scalar_mul(m2[:, :], q8[:, :], 1.0 / H)
    nc.vector.tensor_mul(var[:, :], mu[:, :], mu[:, :])
    nc.vector.tensor_sub(var[:, :], m2[:, :], var[:, :])
    nc.scalar.activation(sd[:, :], var[:, :], AF.Sqrt, bias=EPS)
    nc.vector.reciprocal(a8[:, :], sd[:, :])
    nc.vector.tensor_mul(b28[:, :], mu[:, :], a8[:, :])
    nc.vector.tensor_scalar_mul(b28[:, :], b28[:, :], -1.0)
    return a8, b28


def _ln_bcast(nc, pool, ab, t8):
    """Extract row t8 from the batched (a8,b28) and broadcast to [P,TC]."""
    a8, b28 = ab
    a_row = pool.tile([1, TC], F32, tag="a_row", name="a_row")
    b2_row = pool.tile([1, TC], F32, tag="b2_row", name="b2_row")
    nc.sync.dma_start(out=a_row[:, :], in_=a8[t8:t8 + 1, :])
    nc.sync.dma_start(out=b2_row[:, :], in_=b28[t8:t8 + 1, :])
    a_b = pool.tile([P, TC], F32, tag="a_b", name="a_b")
    b2_b = pool.tile([P, TC], F32, tag="b2_b", name="b2_b")
    nc.gpsimd.partition_broadcast(a_b[:, :], a_row[:, :])
    nc.gpsimd.partition_broadcast(b2_b[:, :], b2_row[:, :])
    return a_b, b2_b


# ----------------------------------------------------------------------
_cache = {}


def _get_program(mask_np, zero_bv, zero_bias):
    key = (mask_np.tobytes(), zero_bv, zero_bias)
    kh = hash(key)
    if kh not in _cache:
        _cache[kh] = build_program(_causal_block_status(mask_np), zero_bv,
                                   zero_bias)
    return _cache[kh]


def kernel(hidden_states, mask, ln1_w, ln1_b, w_qkv, b_qkv, w_dense, b_dense,
           ln3_w, ln3_b, ln2_w, ln2_b, w_h4h, b_h4h, w_4hh, b_4hh,
           ln4_w, ln4_b):
    hidden_states = np.asarray(hidden_states, np.float32)
    mask2d = np.asarray(mask, np.float32).reshape(S, S)
    w_qkv = np.asarray(w_qkv, np.float32)
    b_qkv = np.asarray(b_qkv, np.float32)
    w_dense = np.asarray(w_dense, np.float32)
    w_h4h = np.asarray(w_h4h, np.float32)
    w_4hh = np.asarray(w_4hh, np.float32)

    zero_bv = bool(np.all(b_qkv[2 * H:] == 0.0))
    zero_bias = bool(np.all(b_qkv[:2 * H] == 0.0)
                     and np.all(np.asarray(b_dense) == 0.0)
                     and np.all(np.asarray(b_4hh) == 0.0))
    prog = _get_program(mask2d, zero_bv, zero_bias)

    hT = np.ascontiguousarray(hidden_states.reshape(T, H).T)
    maskT_bf = np.ascontiguousarray(mask2d.T).astype(bf16)

    in_maps = []
    for c in range(NC):
        qs = slice(c * DC, (c + 1) * DC)
        wq_c = np.concatenate([w_qkv[:, c * DC:(c + 1) * DC],
                               w_qkv[:, H + c * DC:H + (c + 1) * DC],
                               w_qkv[:, 2 * H + c * DC:2 * H + (c + 1) * DC]],
                              axis=1)
        b_qk_c = np.concatenate([b_qkv[c * DC:(c + 1) * DC],
                                 b_qkv[H + c * DC:H + (c + 1) * DC]])
        b_v_c = b_qkv[2 * H + c * DC:2 * H + (c + 1) * DC]
        im = {
            "h_ln1": np.ascontiguousarray(hT[:, c * TC:(c + 1) * TC]),
            "h_res": np.ascontiguousarray(hT[qs, :]),
            "ln1_w": np.asarray(ln1_w, np.float32).reshape(H, 1),
            "ln1_b": np.asarray(ln1_b, np.float32).reshape(H, 1),
            "ln2_w": np.asarray(ln2_w, np.float32)[qs].reshape(DC, 1),
            "ln2_b": np.asarray(ln2_b, np.float32)[qs].reshape(DC, 1),
            "ln3_w": np.asarray(ln3_w, np.float32)[qs].reshape(DC, 1),
            "ln3_b": np.asarray(ln3_b, np.float32)[qs].reshape(DC, 1),
            "ln4_w": np.asarray(ln4_w, np.float32)[qs].reshape(DC, 1),
            "ln4_b": np.asarray(ln4_b, np.float32)[qs].reshape(DC, 1),
            "w_qkv": np.ascontiguousarray(wq_c).astype(bf16),
            "b_qk": np.ascontiguousarray(b_qk_c).reshape(2 * DC, 1),
            "b_v": np.ascontiguousarray(b_v_c).reshape(1, DC),
            "w_dense": np.ascontiguousarray(w_dense[:, qs]).astype(bf16),
            "b_dense": np.asarray(b_dense, np.float32)[qs].reshape(DC, 1),
            "w_h4h": np.ascontiguousarray(
                w_h4h[:, c * FC:(c + 1) * FC]).astype(bf16),
            "b_h4h": np.asarray(b_h4h, np.float32)[
                c * FC:(c + 1) * FC].reshape(FC, 1),
            "w_4hh": np.ascontiguousarray(w_4hh[:, qs]).astype(bf16),
            "b_4hh": np.asarray(b_4hh, np.float32)[qs].reshape(DC, 1),
            "maskT": maskT_bf,
        }
        in_maps.append(im)

    res = run_bass_kernel_spmd(prog, in_maps, core_ids=list(range(NC)))
    outT = np.concatenate([res.results[c]["out"] for c in range(NC)], axis=0)
    return np.ascontiguousarray(outT.T).reshape(B, S, H).astype(np.float32)



# revision 17
# speedup vs baseline: 1.5001x; 1.5001x over previous
"""Trainium2 8-core transformer layer — v9: batch x head-group sharding.

Core c = (b, j) with b = c // 2, j = c % 2 handles batch b and head
group j (16 of 32 heads), and owns token half j of batch b (512 tokens)
for the residual / MLP stream.

- LN1 computed locally over the full batch's 1024 tokens (duplicated in
  the pair — cheaper than exchanging x1).
- QKV/attention fully local per core (16 heads x own batch).
- Attention dense projection: row-parallel over the pair; one pairwise
  bf16 ReduceScatter gives each core the summed attn_out for its own
  512 tokens.  That is the ONLY data collective in the kernel.
- LN3 + residual + LN2 + full MLP (h4h, gelu, 4hh) + LN4 + final
  residual all fully local on the own 512 tokens; MLP weights (full
  W_h4h / W_4hh) are streamed from HBM in m-chunk packs.
"""

import os
import sys

sys.path.insert(0, "/opt/trn_rl_repo")
os.environ.setdefault("MYCRO_LOCAL_CACHE", "1")
os.environ.setdefault("JAX_PLATFORMS", "cpu,axon")

import numpy as np
import ml_dtypes

import concourse.bass as bass
import concourse.mybir as mybir
import concourse.tile as tile
from concourse import bacc
from concourse.bass_utils import run_bass_kernel_spmd

F32 = mybir.dt.float32
BF16 = mybir.dt.bfloat16
AF = mybir.ActivationFunctionType
ALU = mybir.AluOpType

P = 128
B, S, H, NH = 4, 1024, 2048, 32
HD = H // NH
NC = 8
HG = NH // 2                   # 16 heads per core
NPR = HG // 2                  # 8 head pairs per core
TOK = 512                      # own tokens per core
SB = 1024                      # batch tokens
TC = 512
NFC = H // P                   # 16
NM = 4 * H // P                # 64 inter chunks
F4 = 4 * H
EPS = 1e-5
RG_PAIR = [[0, 1], [2, 3], [4, 5], [6, 7]]

bf16 = ml_dtypes.bfloat16


def _causal_block_status(mask2d):
    mt = mask2d.T
    status = {}
    for kt in range(S // P):
        for qc in range(S // TC):
            blk = mt[kt * P:(kt + 1) * P, qc * TC:(qc + 1) * TC]
            if np.all(blk == 0):
                status[(kt, qc)] = "skip"
            elif np.all(blk == 1):
                status[(kt, qc)] = "full"
            else:
                status[(kt, qc)] = "masked"
    return status


def build_program(block_status, zero_bv=True, zero_bias=True):
    nc = bacc.Bacc("TRN2", target_bir_lowering=False, debug=False,
                   num_devices=NC)

    def register_const_ap(dtype, value):
        t = nc.alloc_sbuf_tensor(f"const-{dtype.name}-{value}", [128, 1], dtype)
        nc.gpsimd.memset(t.ap(), value)
        nc.const_aps.aps[(dtype, value)] = t.ap()

    register_const_ap(F32, EPS)
    register_const_ap(F32, float(1.0 / np.sqrt(HD)))
    nc.all_engine_barrier()

    # ---------------- DRAM I/O ----------------
    h_batch = nc.dram_tensor("h_batch", [H, SB], BF16, kind="ExternalInput")
    h_own = nc.dram_tensor("h_own", [H, TOK], BF16, kind="ExternalInput")
    # cpack columns: ln1w 0:16, ln1b 16:32, ln2w 32:48, ln2b 48:64,
    # ln3w 64:80, ln3b 80:96, ln4w 96:112, ln4b 112:128,
    # b_qk 128:144 (m-chunks), b_h4h 144:208 (m), b_4hh 208:224,
    # b_dense 224:240
    cpack_d = nc.dram_tensor("cpack", [P, 240], F32, kind="ExternalInput")
    b_v = nc.dram_tensor("b_v", [1, HG * HD], F32, kind="ExternalInput")
    wqk_d = nc.dram_tensor("wqk", [P, 16 * NFC * P], BF16,
                           kind="ExternalInput")
    wv_d = nc.dram_tensor("wv", [P, 2 * NFC * TC], BF16,
                          kind="ExternalInput")
    wd_d = nc.dram_tensor("wd", [P, NFC * 8 * P], BF16, kind="ExternalInput")
    w4h_d = nc.dram_tensor("w4h", [P, NM * NFC * P], BF16,
                           kind="ExternalInput")
    w4hh_d = nc.dram_tensor("w4hh", [P, NFC * NM * P], BF16,
                            kind="ExternalInput")
    mask_d = nc.dram_tensor("maskb", [P, 8 * TC], BF16, kind="ExternalInput")
    out_ext = nc.dram_tensor("out", [H, TOK], F32, kind="ExternalOutput")

    masked_blocks = sorted(k for k, v in block_status.items()
                           if v == "masked")
    mask_slot = {blk: i for i, blk in enumerate(masked_blocks)}
    assert len(masked_blocks) <= 8

    with tile.TileContext(nc) as tc:
        with tc.tile_pool(name="const", bufs=1) as const, \
             tc.tile_pool(name="resid", bufs=1) as resid, \
             tc.tile_pool(name="dram", bufs=1, space="DRAM") as dram:

            # ---------- constants ----------
            ones_bf = const.tile([P, 1], BF16)
            nc.vector.memset(ones_bf[:, :], 1.0)
            ones_rows_bf = const.tile([P, P], BF16)
            nc.vector.memset(ones_rows_bf[:, :], 1.0)

            cpack = const.tile([P, 240], F32)
            nc.sync.dma_start(out=cpack[:, :], in_=cpack_d[:, :])
            ln1w, ln1b = cpack[:, 0:16], cpack[:, 16:32]
            ln2w, ln2b = cpack[:, 32:48], cpack[:, 48:64]
            ln3w, ln3b = cpack[:, 64:80], cpack[:, 80:96]
            ln4w, ln4b = cpack[:, 96:112], cpack[:, 112:128]
            bqk = cpack[:, 128:144]
            bh4h = cpack[:, 144:208]
            b4hh = cpack[:, 208:224]
            bdense = cpack[:, 224:240]

            if not zero_bv:
                bv_row = const.tile([1, HG * HD], F32)
                nc.sync.dma_start(out=bv_row[:, :], in_=b_v[0:1, :])
                bv_b = const.tile([P, HG * HD], F32)
                nc.gpsimd.partition_broadcast(bv_b[:, :], bv_row[:, :])

            mask_sb = const.tile([P, 8 * TC], BF16)
            nc.sync.dma_start(out=mask_sb[:, :], in_=mask_d[:, :])

            # ---------- DRAM bounces ----------
            rs_in = dram.tile([2 * P, NFC * TC], BF16, name="rs_in")
            rs_out = dram.tile([P, NFC * TC], BF16, name="rs_out")
            warm_in = dram.tile([2, 64], BF16, name="warm_in")
            warm_out = dram.tile([1, 64], BF16, name="warm_out")
            warm_sb = const.tile([2, 64], BF16)
            nc.vector.memset(warm_sb[:, :], 0.0)
            nc.sync.dma_start(out=warm_in[:, :], in_=warm_sb[:, :])
            nc.gpsimd.collective_compute(
                "ReduceScatter", ALU.add, replica_groups=RG_PAIR,
                ins=[warm_in[:, :].opt()], outs=[warm_out[:, :].opt()])

            # ---------- residents ----------
            h_res = resid.tile([P, NFC * TOK], BF16, name="h_res")
            for fc in range(NFC):
                nc.sync.dma_start(out=h_res[:, fc * TOK:(fc + 1) * TOK],
                                  in_=h_own[fc * P:(fc + 1) * P, :])
            ln_in = resid.tile([P, NFC * TOK], BF16, name="ln_in")
            mlp_sb = resid.tile([P, NFC * TOK], BF16, name="mlp_sb")

            # =========================================================
            # Phase 1+2 per token-half q: LN1 -> x1; QK (streamed w);
            # V (streamed w, transposed into v_sb)
            # =========================================================
            p1_cm = tc.tile_pool(name="p1", bufs=1)
            p1 = p1_cm.__enter__()
            x1 = p1.tile([P, NFC * SB], BF16, name="x1")
            qT = p1.tile([P, NPR * SB], BF16, name="qT")
            kT = p1.tile([P, NPR * SB], BF16, name="kT")
            v_sb = p1.tile([P, 8 * HG * HD], BF16, name="v_sb")
            ctxF = p1.tile([P, NPR * SB], BF16, name="ctxF")

            with tc.tile_pool(name="ph1", bufs=1) as ph1, \
                 tc.tile_pool(name="ph1ps", bufs=1, space="PSUM") as ph1ps:
                for q in range(2):
                    # ---- LN1 for tokens q*512..(q+1)*512 ----
                    hq = [ph1.tile([P, TC], BF16, tag=f"hq{fc}", bufs=1,
                                   name=f"hq{fc}") for fc in range(NFC)]
                    for fc in range(NFC):
                        nc.sync.dma_start(
                            out=hq[fc][:, :],
                            in_=h_batch[fc * P:(fc + 1) * P,
                                        q * TC:(q + 1) * TC])
                    ps_s = ph1ps.tile([1, TC], F32, tag="st_s", bufs=1,
                                      name="ps_s")
                    ps_q = ph1ps.tile([1, TC], F32, tag="st_q", bufs=1,
                                      name="ps_q")
                    for fc in range(NFC):
                        nc.tensor.matmul(ps_s[:, :], ones_bf[:, 0:1],
                                         hq[fc][:, :], start=(fc == 0),
                                         stop=(fc == NFC - 1))
                        sq = ph1.tile([P, TC], BF16, tag="sq", bufs=3,
                                      name="sq")
                        nc.vector.tensor_mul(sq[:, :], hq[fc][:, :],
                                             hq[fc][:, :])
                        nc.tensor.matmul(ps_q[:, :], ones_bf[:, 0:1],
                                         sq[:, :], start=(fc == 0),
                                         stop=(fc == NFC - 1))
                    a_b, b2_b = _ln_rows(nc, ph1, ps_s, ps_q, f"ln1q{q}", H)
                    for fc in range(NFC):
                        t1 = ph1.tile([P, TC], F32, tag="t1", bufs=2,
                                      name="t1")
                        nc.vector.tensor_mul(t1[:, :], hq[fc][:, :],
                                             a_b[:, :])
                        nc.vector.tensor_add(t1[:, :], t1[:, :], b2_b[:, :])
                        nc.vector.tensor_scalar(
                            x1[:, fc * SB + q * TC:fc * SB + (q + 1) * TC],
                            t1[:, :], ln1w[:, fc:fc + 1], ln1b[:, fc:fc + 1],
                            ALU.mult, ALU.add)

                    # ---- QK for this half ----
                    for m in range(16):
                        wt = ph1.tile([P, NFC * P], BF16, tag="wqk", bufs=2,
                                      name="wqk")
                        nc.sync.dma_start(
                            out=wt[:, :],
                            in_=wqk_d[:, m * NFC * P:(m + 1) * NFC * P])
                        ps = ph1ps.tile([P, TC], F32, tag="qk", bufs=2,
                                        name="ps_qk")
                        for fc in range(NFC):
                            nc.tensor.matmul(
                                ps[:, :], wt[:, fc * P:(fc + 1) * P],
                                x1[:, fc * SB + q * TC:fc * SB + (q + 1) * TC],
                                start=(fc == 0), stop=(fc == NFC - 1))
                        dst = qT if m < 8 else kT
                        pr = m % 8
                        off = pr * SB + q * TC
                        if zero_bias:
                            nc.scalar.activation(dst[:, off:off + TC],
                                                 ps[:, :], AF.Copy)
                        else:
                            nc.scalar.activation(dst[:, off:off + TC],
                                                 ps[:, :], AF.Identity,
                                                 bias=bqk[:, m:m + 1])

                    # ---- V for this half (psum per 128-token block) ----
                    for vf in range(2):
                        psv = [ph1ps.tile([P, TC], F32, tag=f"v{tt}", bufs=1,
                                          name=f"ps_v{tt}")
                               for tt in range(4)]
                        for fc in range(NFC):
                            wvt = ph1.tile([P, TC], BF16, tag="wv", bufs=3,
                                           name="wvt")
                            nc.sync.dma_start(
                                out=wvt[:, :],
                                in_=wv_d[:, (vf * NFC + fc) * TC:
                                         (vf * NFC + fc + 1) * TC])
                            for tt in range(4):
                                nc.tensor.matmul(
                                    psv[tt][:, :],
                                    x1[:, fc * SB + q * TC + tt * P:
                                       fc * SB + q * TC + (tt + 1) * P],
                                    wvt[:, :],
                                    start=(fc == 0), stop=(fc == NFC - 1))
                        for tt in range(4):
                            voff = (q * 4 + tt) * HG * HD + vf * TC
                            if zero_bv:
                                nc.scalar.activation(v_sb[:, voff:voff + TC],
                                                     psv[tt][:, :], AF.Copy)
                            else:
                                nc.vector.tensor_add(
                                    v_sb[:, voff:voff + TC], psv[tt][:, :],
                                    bv_b[:, vf * TC:(vf + 1) * TC])
            # =========================================================
            # Phase 3: attention, 16 heads in 4 quad-groups
            # =========================================================
            with tc.tile_pool(name="ph3", bufs=1) as ph3, \
                 tc.tile_pool(name="ph3ps", bufs=1, space="PSUM") as ph3ps:
                for qc in range(2):
                    kts = [kt for kt in range(S // P)
                           if block_status[(kt, qc)] != "skip"]
                    nkt = len(kts)
                    for hg in range(4):
                        ctx_ps = [ph3ps.tile([P, TC], F32, tag=f"ctx{p2}",
                                             bufs=1, name=f"ctx_ps{p2}")
                                  for p2 in range(2)]
                        den_ps = ph3ps.tile([P, TC], F32, tag="den", bufs=1,
                                            name="den_ps")

                        def emit_scores(ki):
                            kt = kts[ki]
                            st = block_status[(kt, qc)]
                            es = []
                            for i in range(4):
                                h = hg * 4 + i
                                pr, rho = h // 2, h % 2
                                ps_s = ph3ps.tile([P, TC], F32, tag="s",
                                                  bufs=4, name="ps_s")
                                nc.tensor.matmul(
                                    ps_s[:, :],
                                    kT[rho * HD:(rho + 1) * HD,
                                       pr * SB + kt * P:pr * SB + (kt + 1) * P],
                                    qT[rho * HD:(rho + 1) * HD,
                                       pr * SB + qc * TC:pr * SB + (qc + 1) * TC],
                                    start=True, stop=True)
                                e = ph3.tile([P, TC], BF16, tag="e", bufs=10,
                                             name="e")
                                nc.scalar.activation(e[:, :], ps_s[:, :],
                                                     AF.Exp,
                                                     scale=1.0 / np.sqrt(HD))
                                if st == "masked":
                                    i_m = mask_slot[(kt, qc)]
                                    nc.vector.tensor_mul(
                                        e[:, :], e[:, :],
                                        mask_sb[:, i_m * TC:(i_m + 1) * TC])
                                es.append(e)
                            return es

                        def emit_ctx(ki, es):
                            kt = kts[ki]
                            for i in range(4):
                                h = hg * 4 + i
                                pl, rho = i // 2, i % 2
                                nc.tensor.matmul(
                                    ctx_ps[pl][rho * HD:(rho + 1) * HD, :],
                                    v_sb[:, kt * HG * HD + h * HD:
                                         kt * HG * HD + (h + 1) * HD],
                                    es[i][:, :],
                                    start=(ki == 0), stop=(ki == nkt - 1))
                                nc.tensor.matmul(
                                    den_ps[32 * i:32 * i + 1, :],
                                    ones_bf[:, 0:1], es[i][:, :],
                                    start=(ki == 0), stop=(ki == nkt - 1),
                                    tile_position=(0, 32 * i))

                        prev = emit_scores(0)
                        for ki in range(1, nkt):
                            cur = emit_scores(ki)
                            emit_ctx(ki - 1, prev)
                            prev = cur
                        emit_ctx(nkt - 1, prev)

                        # normalize: batched reciprocal of 4 head denoms
                        rd = ph3.tile([P, TC], F32, tag="rd", bufs=2,
                                      name="rd")
                        rd_bf = ph3.tile([P, TC], BF16, tag="rd_bf", bufs=2,
                                         name="rd_bf")
                        nc.vector.reciprocal(rd[0:97, :], den_ps[0:97, :])
                        nc.vector.tensor_copy(rd_bf[0:97, :], rd[0:97, :])
                        for i in range(4):
                            h = hg * 4 + i
                            pr, rho = h // 2, h % 2
                            pl = i // 2
                            r32 = slice(32 * i, 32 * i + 1)
                            ps_b = ph3ps.tile([P, TC], F32, tag="s", bufs=4,
                                              name="ps_b")
                            nc.tensor.matmul(ps_b[:, :],
                                             ones_rows_bf[r32, :],
                                             rd_bf[r32, :], start=True,
                                             stop=True,
                                             tile_position=(32 * i, 0))
                            rd_b = ph3.tile([P, TC], F32, tag="rd_b", bufs=2,
                                            name="rd_b")
                            nc.vector.tensor_copy(rd_b[:, :], ps_b[:, :])
                            hs = slice(rho * HD, (rho + 1) * HD)
                            nc.vector.tensor_mul(
                                ctxF[hs, pr * SB + qc * TC:
                                     pr * SB + (qc + 1) * TC],
                                ctx_ps[pl][hs, :], rd_b[hs, :])

            # =========================================================
            # Phase 4: dense partial -> pairwise ReduceScatter
            # =========================================================
            with tc.tile_pool(name="ph4", bufs=1) as ph4, \
                 tc.tile_pool(name="ph4ps", bufs=1, space="PSUM") as ph4ps:
                for fco in range(NFC):
                    wt = ph4.tile([P, 8 * P], BF16, tag="wd", bufs=3,
                                  name="wd")
                    nc.sync.dma_start(
                        out=wt[:, :], in_=wd_d[:, fco * 8 * P:(fco + 1) * 8 * P])
                    for hh in range(2):
                        ps = ph4ps.tile([P, TC], F32, tag="d", bufs=3,
                                        name="ps_d")
                        for kc in range(8):
                            nc.tensor.matmul(
                                ps[:, :], wt[:, kc * P:(kc + 1) * P],
                                ctxF[:, kc * SB + hh * TC:
                                     kc * SB + (hh + 1) * TC],
                                start=(kc == 0), stop=(kc == 7))
                        db = ph4.tile([P, TC], BF16, tag="db", bufs=4,
                                      name="db")
                        nc.scalar.activation(db[:, :], ps[:, :], AF.Copy)
                        nc.sync.dma_start(
                            out=rs_in[hh * P:(hh + 1) * P,
                                      fco * TC:(fco + 1) * TC],
                            in_=db[:, :])
                nc.gpsimd.collective_compute(
                    "ReduceScatter", ALU.add, replica_groups=RG_PAIR,
                    ins=[rs_in[:, :].opt()], outs=[rs_out[:, :].opt()])
            p1_cm.__exit__(None, None, None)

            # =========================================================
            # Phase 5: LN3 + residual -> ln_in; LN2 -> x2
            # =========================================================
            p2_cm = tc.tile_pool(name="p2", bufs=1)
            p2 = p2_cm.__enter__()
            x2 = p2.tile([P, NFC * TOK], BF16, name="x2")
            inter = p2.tile([P, NM * TC], BF16, name="inter")
            with tc.tile_pool(name="ph5", bufs=1) as ph5, \
                 tc.tile_pool(name="ph5ps", bufs=1, space="PSUM") as ph5ps:
                at = [ph5.tile([P, TC], BF16, tag=f"at{fc}", bufs=1,
                               name=f"at{fc}") for fc in range(NFC)]
                for fc in range(NFC):
                    nc.sync.dma_start(out=at[fc][:, :],
                                      in_=rs_out[:, fc * TC:(fc + 1) * TC])
                if not zero_bias:
                    for fc in range(NFC):
                        nc.vector.tensor_scalar_add(at[fc][:, :], at[fc][:, :],
                                                    bdense[:, fc:fc + 1])
                ps_s3 = ph5ps.tile([1, TC], F32, tag="s3", bufs=1,
                                   name="ps_s3")
                ps_q3 = ph5ps.tile([1, TC], F32, tag="q3", bufs=1,
                                   name="ps_q3")
                for fc in range(NFC):
                    nc.tensor.matmul(ps_s3[:, :], ones_bf[:, 0:1],
                                     at[fc][:, :], start=(fc == 0),
                                     stop=(fc == NFC - 1))
                    sq = ph5.tile([P, TC], BF16, tag="sq", bufs=2, name="sq")
                    nc.vector.tensor_mul(sq[:, :], at[fc][:, :], at[fc][:, :])
                    nc.tensor.matmul(ps_q3[:, :], ones_bf[:, 0:1], sq[:, :],
                                     start=(fc == 0), stop=(fc == NFC - 1))
                a3_b, b23_b = _ln_rows(nc, ph5, ps_s3, ps_q3, "ln3", H)
                ps_s2 = ph5ps.tile([1, TC], F32, tag="s2", bufs=1,
                                   name="ps_s2")
                ps_q2 = ph5ps.tile([1, TC], F32, tag="q2", bufs=1,
                                   name="ps_q2")
                for fc in range(NFC):
                    sl = slice(fc * TOK, (fc + 1) * TOK)
                    t1 = ph5.tile([P, TC], F32, tag="t1", bufs=2, name="t1")
                    t2 = ph5.tile([P, TC], F32, tag="t2", bufs=2, name="t2")
                    nc.vector.tensor_mul(t1[:, :], at[fc][:, :], a3_b[:, :])
                    nc.vector.tensor_add(t1[:, :], t1[:, :], b23_b[:, :])
                    nc.vector.tensor_scalar(t2[:, :], t1[:, :],
                                            ln3w[:, fc:fc + 1],
                                            ln3b[:, fc:fc + 1],
                                            ALU.mult, ALU.add)
                    nc.vector.tensor_add(ln_in[:, sl], t2[:, :],
                                         h_res[:, sl])
                    nc.tensor.matmul(ps_s2[:, :], ones_bf[:, 0:1],
                                     ln_in[:, sl], start=(fc == 0),
                                     stop=(fc == NFC - 1))
                    sq = ph5.tile([P, TC], BF16, tag="sq", bufs=2, name="sq")
                    nc.vector.tensor_mul(sq[:, :], ln_in[:, sl], ln_in[:, sl])
                    nc.tensor.matmul(ps_q2[:, :], ones_bf[:, 0:1], sq[:, :],
                                     start=(fc == 0), stop=(fc == NFC - 1))
                a2_b, b22_b = _ln_rows(nc, ph5, ps_s2, ps_q2, "ln2", H)
                for fc in range(NFC):
                    sl = slice(fc * TOK, (fc + 1) * TOK)
                    t1 = ph5.tile([P, TC], F32, tag="t1", bufs=2, name="t1")
                    nc.vector.tensor_mul(t1[:, :], ln_in[:, sl], a2_b[:, :])
                    nc.vector.tensor_add(t1[:, :], t1[:, :], b22_b[:, :])
                    nc.vector.tensor_scalar(x2[:, sl], t1[:, :],
                                            ln2w[:, fc:fc + 1],
                                            ln2b[:, fc:fc + 1],
                                            ALU.mult, ALU.add)

            # =========================================================
            # Phase 6: MLP h4h + gelu -> inter; 4hh -> mlp_sb (+LN4 stats)
            # =========================================================
            with tc.tile_pool(name="ph6", bufs=1) as ph6, \
                 tc.tile_pool(name="ph6ps", bufs=1, space="PSUM") as ph6ps:
                for m in range(NM):
                    wt = ph6.tile([P, NFC * P], BF16, tag="wh", bufs=3,
                                  name="wh")
                    nc.sync.dma_start(
                        out=wt[:, :],
                        in_=w4h_d[:, m * NFC * P:(m + 1) * NFC * P])
                    ps = ph6ps.tile([P, TC], F32, tag="h", bufs=2,
                                    name="ps_h")
                    for fc in range(NFC):
                        nc.tensor.matmul(ps[:, :], wt[:, fc * P:(fc + 1) * P],
                                         x2[:, fc * TOK:(fc + 1) * TOK],
                                         start=(fc == 0),
                                         stop=(fc == NFC - 1))
                    nc.scalar.activation(inter[:, m * TC:(m + 1) * TC],
                                         ps[:, :], AF.Gelu_apprx_tanh,
                                         bias=bh4h[:, m:m + 1])
                ps_s4 = ph6ps.tile([1, TC], F32, tag="s4", bufs=1,
                                   name="ps_s4")
                ps_q4 = ph6ps.tile([1, TC], F32, tag="q4", bufs=1,
                                   name="ps_q4")
                for fco in range(NFC):
                    wt = [ph6.tile([P, NM * P // 2], BF16, tag="w4",
                                   bufs=3, name="w4")
                          for half in range(2)]
                    for half in range(2):
                        nc.sync.dma_start(
                            out=wt[half][:, :],
                            in_=w4hh_d[:, (fco * NM + half * NM // 2) * P:
                                       (fco * NM + (half + 1) * NM // 2) * P])
                    ps = ph6ps.tile([P, TC], F32, tag="f", bufs=2,
                                    name="ps_f")
                    for kc in range(NM):
                        half, kk = kc // (NM // 2), kc % (NM // 2)
                        nc.tensor.matmul(ps[:, :],
                                         wt[half][:, kk * P:(kk + 1) * P],
                                         inter[:, kc * TC:(kc + 1) * TC],
                                         start=(kc == 0),
                                         stop=(kc == NM - 1))
                    sl = slice(fco * TOK, (fco + 1) * TOK)
                    if zero_bias:
                        nc.scalar.activation(mlp_sb[:, sl], ps[:, :], AF.Copy)
                    else:
                        nc.scalar.activation(mlp_sb[:, sl], ps[:, :],
                                             AF.Identity,
                                             bias=b4hh[:, fco:fco + 1])
                    nc.tensor.matmul(ps_s4[:, :], ones_bf[:, 0:1],
                                     mlp_sb[:, sl], start=(fco == 0),
                                     stop=(fco == NFC - 1))
                    sq = ph6.tile([P, TC], BF16, tag="sq", bufs=2, name="sq")
                    nc.vector.tensor_mul(sq[:, :], mlp_sb[:, sl],
                                         mlp_sb[:, sl])
                    nc.tensor.matmul(ps_q4[:, :], ones_bf[:, 0:1], sq[:, :],
                                     start=(fco == 0), stop=(fco == NFC - 1))

                # LN4 + final residual -> out
                a4_b, b24_b = _ln_rows(nc, ph6, ps_s4, ps_q4, "ln4", H)
                for fc in range(NFC):
                    sl = slice(fc * TOK, (fc + 1) * TOK)
                    t1 = ph6.tile([P, TC], F32, tag="t1", bufs=2, name="t1")
                    t2 = ph6.tile([P, TC], F32, tag="t2", bufs=2, name="t2")
                    nc.vector.tensor_mul(t1[:, :], mlp_sb[:, sl], a4_b[:, :])
                    nc.vector.tensor_add(t1[:, :], t1[:, :], b24_b[:, :])
                    nc.vector.tensor_scalar(t2[:, :], t1[:, :],
                                            ln4w[:, fc:fc + 1],
                                            ln4b[:, fc:fc + 1],
                                            ALU.mult, ALU.add)
                    ot = ph6.tile([P, TC], F32, tag="ot", bufs=2, name="ot")
                    nc.vector.tensor_add(ot[:, :], t2[:, :], ln_in[:, sl])
                    nc.sync.dma_start(out=out_ext[fc * P:(fc + 1) * P, :],
                                      in_=ot[:, :])
            p2_cm.__exit__(None, None, None)

    nc.compile()
    return nc


def _ln_rows(nc, pool, ps_s, ps_q, name, nfeat, bbufs=1):
    """LN row math on [1,TC] stat psums -> broadcast a_b, b2_b [P,TC]."""
    mu = pool.tile([1, TC], F32, tag="lnr_t1", bufs=1, name=f"{name}_mu")
    m2 = pool.tile([1, TC], F32, tag="lnr_t2", bufs=1, name=f"{name}_m2")
    var = pool.tile([1, TC], F32, tag="lnr_t3", bufs=1, name=f"{name}_var")
    sd = pool.tile([1, TC], F32, tag="lnr_t2", bufs=1, name=f"{name}_sd")
    a_row = pool.tile([1, TC], F32, tag="lnr_t3", bufs=1, name=f"{name}_a")
    b2_row = pool.tile([1, TC], F32, tag="lnr_t2", bufs=1, name=f"{name}_b2")
    nc.vector.tensor_scalar_mul(mu[:, :], ps_s[:, :], 1.0 / nfeat)
    nc.vector.tensor_scalar_mul(m2[:, :], ps_q[:, :], 1.0 / nfeat)
    nc.vector.tensor_mul(var[:, :], mu[:, :], mu[:, :])
    nc.vector.tensor_sub(var[:, :], m2[:, :], var[:, :])
    nc.scalar.activation(sd[:, :], var[:, :], AF.Sqrt, bias=EPS)
    nc.vector.reciprocal(a_row[:, :], sd[:, :])
    nc.vector.tensor_mul(b2_row[:, :], mu[:, :], a_row[:, :])
    nc.vector.tensor_scalar_mul(b2_row[:, :], b2_row[:, :], -1.0)
    a_b = pool.tile([P, TC], F32, tag="lnr_ab", bufs=bbufs, name=f"{name}_ab")
    b2_b = pool.tile([P, TC], F32, tag="lnr_b2b", bufs=bbufs,
                     name=f"{name}_b2b")
    nc.gpsimd.partition_broadcast(a_b[:, :], a_row[:, :])
    nc.gpsimd.partition_broadcast(b2_b[:, :], b2_row[:, :])
    return a_b, b2_b


# ----------------------------------------------------------------------
_cache = {}


def _get_program(mask_np, zero_bv, zero_bias):
    key = (mask_np.tobytes(), zero_bv, zero_bias)
    kh = hash(key)
    if kh not in _cache:
        _cache[kh] = build_program(_causal_block_status(mask_np), zero_bv,
                                   zero_bias)
    return _cache[kh]


def kernel(hidden_states, mask, ln1_w, ln1_b, w_qkv, b_qkv, w_dense, b_dense,
           ln3_w, ln3_b, ln2_w, ln2_b, w_h4h, b_h4h, w_4hh, b_4hh,
           ln4_w, ln4_b):
    hidden_states = np.asarray(hidden_states, np.float32)
    mask2d = np.asarray(mask, np.float32).reshape(S, S)
    w_qkv = np.asarray(w_qkv, np.float32)
    b_qkv = np.asarray(b_qkv, np.float32)
    w_dense = np.asarray(w_dense, np.float32)
    b_dense = np.asarray(b_dense, np.float32)
    w_h4h = np.asarray(w_h4h, np.float32)
    b_h4h = np.asarray(b_h4h, np.float32)
    w_4hh = np.asarray(w_4hh, np.float32)
    b_4hh = np.asarray(b_4hh, np.float32)

    zero_bv = bool(np.all(b_qkv[2 * H:] == 0.0))
    zero_bias = bool(np.all(b_qkv[:2 * H] == 0.0)
                     and np.all(b_dense == 0.0)
                     and np.all(b_4hh == 0.0))
    prog = _get_program(mask2d, zero_bv, zero_bias)

    block_status = _causal_block_status(mask2d)
    masked_blocks = sorted(k for k, v in block_status.items()
                           if v == "masked")
    mask_pack = np.zeros((P, 8 * TC), np.float32)
    mt = mask2d.T
    for i, (kt, qc) in enumerate(masked_blocks):
        mask_pack[:, i * TC:(i + 1) * TC] = \
            mt[kt * P:(kt + 1) * P, qc * TC:(qc + 1) * TC]
    mask_pack = mask_pack.astype(bf16)

    # shared weight packs (same for all cores)
    w4h_pack = np.ascontiguousarray(
        w_h4h.reshape(NFC, P, NM, P).transpose(1, 2, 0, 3)
        .reshape(P, NM * NFC * P)).astype(bf16)
    w4hh_pack = np.ascontiguousarray(
        w_4hh.reshape(NM, P, NFC, P).transpose(1, 2, 0, 3)
        .reshape(P, NFC * NM * P)).astype(bf16)

    # per-head-group (j) packs
    wqk_packs, wv_packs, wd_packs, bqk_cols, bv_rows = [], [], [], [], []
    for j in range(2):
        qo, ko, vo = j * 1024, H + j * 1024, 2 * H + j * 1024
        wq = w_qkv[:, qo:qo + 1024]      # [2048, 1024]
        wk = w_qkv[:, ko:ko + 1024]
        wv = w_qkv[:, vo:vo + 1024]
        wqk = np.concatenate([wq, wk], axis=1)   # m-chunks 0..15
        wqk_packs.append(np.ascontiguousarray(
            wqk.reshape(NFC, P, 16, P).transpose(1, 2, 0, 3)
            .reshape(P, 16 * NFC * P)).astype(bf16))
        # wv chunk (vf, fc) at [:, (vf*NFC+fc)*TC : ...]
        wv_packs.append(np.ascontiguousarray(
            wv.reshape(NFC, P, 2, TC).transpose(1, 2, 0, 3)
            .reshape(P, 2 * NFC * TC)).astype(bf16))
        wd = w_dense[j * 1024:(j + 1) * 1024, :]  # [1024, 2048]
        wd_packs.append(np.ascontiguousarray(
            wd.reshape(8, P, NFC, P).transpose(1, 2, 0, 3)
            .reshape(P, NFC * 8 * P)).astype(bf16))
        bq = np.concatenate([b_qkv[qo:qo + 1024], b_qkv[ko:ko + 1024]])
        bqk_cols.append(bq.reshape(16, P).T)     # [128, 16]
        bv_rows.append(b_qkv[vo:vo + 1024].reshape(1, 1024))

    def col16(v):
        return v.reshape(NFC, P).T               # [128, 16]

    cpacks = []
    for j in range(2):
        cp = np.zeros((P, 240), np.float32)
        cp[:, 0:16] = col16(np.asarray(ln1_w, np.float32))
        cp[:, 16:32] = col16(np.asarray(ln1_b, np.float32))
        cp[:, 32:48] = col16(np.asarray(ln2_w, np.float32))
        cp[:, 48:64] = col16(np.asarray(ln2_b, np.float32))
        cp[:, 64:80] = col16(np.asarray(ln3_w, np.float32))
        cp[:, 80:96] = col16(np.asarray(ln3_b, np.float32))
        cp[:, 96:112] = col16(np.asarray(ln4_w, np.float32))
        cp[:, 112:128] = col16(np.asarray(ln4_b, np.float32))
        cp[:, 128:144] = bqk_cols[j]
        cp[:, 144:208] = b_h4h.reshape(NM, P).T
        cp[:, 208:224] = col16(b_4hh)
        cp[:, 224:240] = col16(b_dense)
        cpacks.append(cp)

    h_batches = []
    for b in range(B):
        h_batches.append(np.ascontiguousarray(
            hidden_states[b].T).astype(bf16))    # [2048, 1024] bf16

    in_maps = []
    for c in range(NC):
        b, j = c // 2, c % 2
        h_own = np.ascontiguousarray(
            hidden_states[b, j * TOK:(j + 1) * TOK, :].T).astype(bf16)
        im = {
            "h_batch": h_batches[b],
            "h_own": h_own,
            "cpack": cpacks[j],
            "b_v": bv_rows[j],
            "wqk": wqk_packs[j],
            "wv": wv_packs[j],
            "wd": wd_packs[j],
            "w4h": w4h_pack,
            "w4hh": w4hh_pack,
            "maskb": mask_pack,
        }
        in_maps.append(im)

    res = run_bass_kernel_spmd(prog, in_maps, core_ids=list(range(NC)))
    out = np.empty((B, S, H), np.float32)
    for c in range(NC):
        b, j = c // 2, c % 2
        out[b, j * TOK:(j + 1) * TOK, :] = res.results[c]["out"].T
    return out
